# revision 7
# baseline (speedup 1.0000x reference)
"""MultiHeadAttention (causal + ALiBi) Trainium2 kernel, 8-core SPMD. v2.

Sharding: core c -> batch b = c // 4, head-group j = c % 4 owning global
heads {j, j+4, j+8, j+12} (strided so every core gets one head from each
slope class). Each core projects q/k/v for its 4 heads from x[b], runs
windowed-causal attention in a transposed layout (scores^T[j_kv, i_q]),
and emits a partial out-projection y^T [D, S]. Host sums the 4 partials
per batch plus the constant term (w_out@b_v + b_out) and returns [B,S,D].

v2 changes vs baseline:
- all matmuls in bf16 (q/k/v/probs/attn/wout); PSUM accumulation stays f32.
- much tighter ALiBi j-windows (measured truncation err 2e-3 rel, gate 2e-2).
- slot0 widened to W=128 with mid-chunk exp re-centering (bias
  slope*(p - o - W/2)), halving slot0 instruction counts.
- causal masking moved off the PSUM path: probs *= {0,1} bf16 mask on DVE
  (2x mode) after exp instead of adding -1e30 to f32 scores.
- out-projection emitted transposed (y^T[feat, tok]) with the bias folded
  into a host-side constant; evacuation split DVE/ACT, bf16 output with
  fb-pair-merged stores (each DMA costs ~625ns of HWDGE regardless of size).
- x/weights streamed as bf16 (host-converted) in few big strided DMAs.
- projection of chunk ch+1 software-pipelined unit-by-unit into the
  attention schedule of chunk ch; i-chunks widened (slot0 128, slots1-3 512
  with valid-column-range partial blocks) to amortize ACT/instr overheads.
"""
import math
from collections import deque
from contextlib import ExitStack

import numpy as np

import concourse.tile as tile
from concourse import bacc, mybir
from concourse.bass_utils import run_bass_kernel_spmd

B, S, D, H, HD = 2, 2048, 1024, 16, 64
N_CORES = 8
DT = mybir.dt
F32, BF16 = DT.float32, DT.bfloat16

SLOT_W = [128, 512, 512, 512]          # i-chunk width per head slot
SLOT_WIN = [40, 112, 384, 1152]        # j-window per slot (max over slot heads)


def slot_blocks(slot):
    """(it, jt, o) list, uniform across cores. o = i0 - 128*jt."""
    W, win = SLOT_W[slot], SLOT_WIN[slot]
    blocks = []
    for it in range(S // W):
        i0 = it * W
        jt_max = (i0 + W - 1) // 128
        jt_min = max(0, math.ceil((i0 - win - 127) / 128))
        for jt in range(jt_min, jt_max + 1):
            blocks.append((it, jt, i0 - 128 * jt))
    return blocks


def slot_offsets(slot):
    return sorted({o for _, _, o in slot_blocks(slot)})


def build_nc(repeat=1):
    nc = bacc.Bacc(
        "TRN2", target_bir_lowering=False, debug=False,
        enable_asserts=False, num_devices=N_CORES,
    )
    dram = {}

    def din(name, shape, dtype):
        dram[name] = nc.dram_tensor(name, shape, dtype, kind="ExternalInput").ap()
        return dram[name]

    din("xT", [D, S], BF16)
    din("wqT", [D, 256], BF16)
    din("wkT", [D, 256], BF16)
    din("wvT", [D, 256], BF16)
    din("bqk_p", [128, 4], F32)           # bq ft0/ft1, bk ft0/ft1 columns
    din("maskpack", [128, 512], BF16)     # {0,1} causal p <= f
    nbtot = sum(len(slot_offsets(s)) for s in range(4))
    din("bias_all", [128, nbtot], F32)
    din("woutT", [2, 128, D], BF16)       # [hd half, 128 hd, feat]
    y_out = nc.dram_tensor("y", [D, S], BF16, kind="ExternalOutput").ap()

    with tile.TileContext(nc) as tc:
        for _ in range(repeat):
            build_body(tc, dram, y_out)
    nc.compile()
    return nc


def build_body(tc, dram, y_out):
    nc = tc.nc
    Exp = mybir.ActivationFunctionType.Exp
    with ExitStack() as ctx:
        consts = ctx.enter_context(tc.tile_pool(name="consts", bufs=1))
        qkpool = ctx.enter_context(tc.tile_pool(name="qk", bufs=1))
        vpool = ctx.enter_context(tc.tile_pool(name="vp", bufs=1))
        attnp = ctx.enter_context(tc.tile_pool(name="attn", bufs=1))
        xtp = ctx.enter_context(tc.tile_pool(name="xt", bufs=3))
        wp = ctx.enter_context(tc.tile_pool(name="w", bufs=3))
        rowp = ctx.enter_context(tc.tile_pool(name="rows", bufs=1))
        prp = ctx.enter_context(tc.tile_pool(name="probs", bufs=16))
        lp = ctx.enter_context(tc.tile_pool(name="lvec", bufs=4))
        rbp = ctx.enter_context(tc.tile_pool(name="rbc", bufs=3))
        yp = ctx.enter_context(tc.tile_pool(name="ysb", bufs=3))
        # PSUM budget (8 banks): big(qkv)=2, y=2, sc=2, pv=2
        big_ps = ctx.enter_context(tc.tile_pool(name="big_ps", bufs=2, space="PSUM"))
        y_ps = ctx.enter_context(tc.tile_pool(name="y_ps", bufs=1, space="PSUM"))
        sc_ps = ctx.enter_context(tc.tile_pool(name="sc_ps", bufs=3, space="PSUM"))
        pv_ps = ctx.enter_context(tc.tile_pool(name="pv_ps", bufs=2, space="PSUM"))

        # ---- persistent q/k/v/attn tiles (slot pairs in partition halves) ----
        q_p = [qkpool.tile([128, S], BF16, tag=f"qp{i}", name=f"qp{i}") for i in range(2)]
        k_p = [qkpool.tile([128, S], BF16, tag=f"kp{i}", name=f"kp{i}") for i in range(2)]
        # V' [128 kv, 16 j-tiles, 4 slots, 65]: 64 value cols + ones col
        v_all = vpool.tile([128, 16, 4, 65], BF16, tag="vall", name="vall")
        attn_sb = [attnp.tile([128, S], BF16, tag=f"attn{i}", name=f"attn{i}") for i in range(2)]

        # ---- phase-A weights (startup critical path) ----
        w_sb = {}
        for nm, dr, eng in (("q", "wqT", nc.scalar), ("k", "wkT", nc.scalar),
                            ("v", "wvT", nc.scalar)):
            t = wp.tile([128, 8, 256], BF16, tag=f"w{nm}", name=f"w{nm}")
            wr = dram[dr].rearrange("(kt p) f -> p kt f", p=128)
            eng.dma_start(out=t[:, 0:4, :], in_=wr[:, 0:4, :])
            eng.dma_start(out=t[:, 4:8, :], in_=wr[:, 4:8, :])
            w_sb[nm] = t
        bcols = rowp.tile([128, 4], F32, tag="bcols", name="bcols")
        nc.sync.dma_start(out=bcols[:], in_=dram["bqk_p"])
        bpair = {"q": [bcols[:, 0:1], bcols[:, 1:2]],
                 "k": [bcols[:, 2:3], bcols[:, 3:4]]}
        nc.vector.memset(v_all[:, :, :, 64:65], 1.0)

        # ---- constants ----
        maskpack = consts.tile([128, 512], BF16, tag="maskpack", name="maskpack")
        nc.scalar.dma_start(out=maskpack[:], in_=dram["maskpack"])
        nbtot = sum(len(slot_offsets(s)) for s in range(4))
        bias_tile = consts.tile([128, nbtot], F32, tag="bias_all", name="bias_all")
        nc.sync.dma_start(out=bias_tile[:], in_=dram["bias_all"])
        bias_sb = []
        col = 0
        for s in range(4):
            d = {}
            for o in slot_offsets(s):
                d[o] = bias_tile[:, col:col + 1]
                col += 1
            bias_sb.append(d)
        woutt = consts.tile([128, 2, D], BF16, tag="woutT", name="woutT")
        nc.scalar.dma_start(
            out=woutt[:], in_=dram["woutT"].rearrange("h p f -> p h f"))
        wout_sb = [woutt[:, 0, :], woutt[:, 1, :]]

        by_slot = []
        for s in range(4):
            W, win = SLOT_W[s], SLOT_WIN[s]
            by_it = {}
            for it, jt, o in slot_blocks(s):
                f_lo = (max(0, -o) // 64) * 64
                f_hi = min(W, -((127 - o + win + 1) // -64) * 64)
                if o == 0:
                    f_hi = W
                by_it.setdefault(it, []).append((jt, o, f_lo, f_hi))
            # full-width block first so PV's start=True covers the whole tile
            for blocks in by_it.values():
                blocks.sort(key=lambda b: (b[3] - b[2] != W, b[0]))
            by_slot.append(by_it)

        xT_r = dram["xT"].rearrange("(kt p) s -> p kt s", p=128)
        xt_tiles = {}

        def load_xt(ch):
            t = xtp.tile([128, 8, 512], BF16, tag="xt", name="xt")
            for half in range(2):
                nc.sync.dma_start(
                    out=t[:, half * 4:(half + 1) * 4, :],
                    in_=xT_r[:, half * 4:(half + 1) * 4,
                             ch * 512:(ch + 1) * 512])
            xt_tiles[ch] = t

        def proj_qk_unit(ch, nm, ft):
            xt = xt_tiles[ch]
            sl = slice(ch * 512, (ch + 1) * 512)
            dst = q_p if nm == "q" else k_p
            ps = big_ps.tile([128, 512], F32, tag="big", name="qkv")
            for kt in range(8):
                nc.tensor.matmul(
                    ps[:], w_sb[nm][:, kt, ft * 128:(ft + 1) * 128],
                    xt[:, kt, :], start=(kt == 0), stop=(kt == 7))
            nc.vector.tensor_scalar_add(
                dst[ft][:, sl], ps[:], bpair[nm][ft][:])

        def proj_v_unit(ch, tl):
            xt = xt_tiles[ch]
            tt = ch * 4 + tl
            ps = big_ps.tile([128, 512], F32, tag="big", name="qkvv")
            for kt in range(8):
                nc.tensor.matmul(
                    ps[:, 0:256], xt[:, kt, tl * 128:(tl + 1) * 128],
                    w_sb["v"][:, kt, :], start=(kt == 0), stop=(kt == 7))
            nc.vector.tensor_copy(
                v_all[:, tt:tt + 1, :, 0:64],
                ps[:, 0:256].rearrange("p (a b) -> p a b", a=4))

        def proj_units(ch):
            """Ordered to unblock the next chunk's schedule: slot-pair 1
            (q_p[1]/k_p[1]) first since the chunk list opens with slot 2."""
            us = [("qk", "q", 1), ("qk", "k", 1), ("v", 0, None), ("v", 1, None),
                  ("qk", "q", 0), ("qk", "k", 0), ("v", 2, None), ("v", 3, None)]
            return deque((ch, u) for u in us)

        def emit_unit(ch, u):
            if u[0] == "qk":
                proj_qk_unit(ch, u[1], u[2])
            else:
                proj_v_unit(ch, u[1])

        def emit_proj(ch):
            for _, u in proj_units(ch):
                emit_unit(ch, u)
            xt_tiles.pop(ch)

        def alloc_sc(Wb):
            return sc_ps.tile([128, 512], F32, tag="sc", name="sc")[:, 0:Wb]

        def alloc_pv(W):
            return pv_ps.tile([65, 512], F32, tag="pv", name="pv")[:, 0:W]

        def emit_scores(s, it):
            """Scores + exp (+ diag mask on Pool) for one chunk."""
            W = SLOT_W[s]
            prs = []
            h0 = (s % 2) * 64
            kp_s = k_p[s // 2]
            qp_s = q_p[s // 2]
            for jt, o, f_lo, f_hi in by_slot[s][it]:
                Wb = f_hi - f_lo
                sc = alloc_sc(Wb)
                nc.tensor.matmul(
                    sc[:, 0:Wb], kp_s[h0:h0 + 64, jt * 128:(jt + 1) * 128],
                    qp_s[h0:h0 + 64, it * W + f_lo:it * W + f_hi],
                    start=True, stop=True)
                ptag = "pr_a" if s == 0 else "pr_b"
                pr = prp.tile([128, W], BF16, tag=ptag, name="pr",
                              bufs=(8 if s == 0 else 22))
                nc.scalar.activation(pr[:, 0:Wb], sc[:, 0:Wb], Exp,
                                     bias=bias_sb[s][o][:])
                if o <= 0:  # diagonal block -> zero out kv > q (bf16 2x DVE)
                    nc.vector.tensor_mul(pr[:, 0:Wb], pr[:, 0:Wb],
                                         maskpack[:, 0:Wb])
                prs.append((jt, pr, f_lo, f_hi))
            return prs

        def emit_pv(s, it, prs, nsplit=1):
            """PV accumulation + normalize epilogue for one chunk. nsplit>1
            pipelines the epilogue in column halves (tail latency)."""
            W = SLOT_W[s]
            pv = alloc_pv(W)
            for bi, (jt, pr, f_lo, f_hi) in enumerate(prs):
                nc.tensor.matmul(
                    pv[:, f_lo:f_hi], v_all[:, jt:jt + 1, s:s + 1, :],
                    pr[:, 0:f_hi - f_lo],
                    start=(bi == 0), stop=(bi == len(prs) - 1))
            rr = lp.tile([1, W], F32, tag="rr", name="rr")
            rb = rbp.tile([64, W], F32, tag="rb", name="rb")
            dst = attn_sb[s // 2]
            r0 = (s % 2) * 64
            wsp = W // nsplit
            for sp in range(nsplit):
                csl = slice(sp * wsp, (sp + 1) * wsp)
                nc.vector.reciprocal(rr[:, csl], pv[64:65, csl])
                nc.gpsimd.partition_broadcast(rb[:, csl], rr[:, csl])
                nc.vector.tensor_mul(
                    dst[r0:r0 + 64, it * W + sp * wsp:it * W + (sp + 1) * wsp],
                    pv[0:64, csl], rb[:, csl])

        y_r = y_out.rearrange("(fp p) s -> p fp s", p=128)
        ysb_cur = {}

        def emit_y(g, fb, c0=0, c1=512):
            """y^T out-projection for token group g cols [c0,c1), feat block fb.
            Stores are merged per fb-pair (one DMA per 256 features)."""
            tsl = slice(g * 512 + c0, g * 512 + c1)
            n = c1 - c0
            if fb % 2 == 1:
                py = big_ps.tile([128, 512], F32, tag="big", name="py")
            else:
                py = y_ps.tile([128, 512], F32, tag="py", name="py")
            nc.tensor.matmul(
                py[:, 0:n], wout_sb[0][:, fb * 128:(fb + 1) * 128],
                attn_sb[0][:, tsl], start=True, stop=False)
            nc.tensor.matmul(
                py[:, 0:n], wout_sb[1][:, fb * 128:(fb + 1) * 128],
                attn_sb[1][:, tsl], start=False, stop=True)
            if fb % 2 == 0:
                ysb_cur[0] = yp.tile([128, 2, 512], BF16, tag="ysb", name="ysb")
                nc.vector.tensor_copy(ysb_cur[0][:, 0, 0:n], py[:, 0:n])
            else:
                ysb = ysb_cur[0]
                nc.scalar.activation(ysb[:, 1, 0:n], py[:, 0:n],
                                     mybir.ActivationFunctionType.Copy)
                nc.sync.dma_start(
                    out=y_r[:, fb - 1:fb + 1, tsl], in_=ysb[:, :, 0:n])

        # ---- fused schedule: proj(ch+1) and y(ch-1) interleave into the
        # attention chunks of ch so PE/ACT/DVE stay fed across the timeline.
        # ch0 leads with slot0 so attention starts after only 3 proj units.
        prev = None
        pending_y = deque()
        units = deque()
        load_xt(0)
        load_xt(1)
        for u in (("qk", "q", 0), ("qk", "k", 0), ("v", 0, None)):
            emit_unit(0, u)
        units.extend([(0, ("v", 1, None)), (0, ("v", 2, None)),
                      (0, ("v", 3, None)), (0, ("qk", "q", 1)),
                      (0, ("qk", "k", 1))])
        units.extend(proj_units(1))
        # units popped per (ch, idx): chunk (s,it) must come after every proj
        # unit it reads -- read-before-write has NO Tile dependency and the
        # scheduler may order it either way (nondeterministic garbage).
        POPS0 = [2, 2, 2, 2, 2, 2, 1]
        POPS = [2, 1, 1, 1, 1, 1, 1, 0, 0, 0]
        for ch in range(4):
            c0 = ch * 4          # slot0 chunks (W=128): c0..c0+3
            if ch == 0:
                chunks = [(0, 0), (0, 1), (0, 2), (0, 3), (2, 0), (3, 0), (1, 0)]
            elif ch < 3:
                chunks = [(2, ch), (0, c0), (3, ch), (0, c0 + 1),
                          (0, c0 + 2), (1, ch), (0, c0 + 3)]
                units.extend(proj_units(ch + 1))
                if ch == 2:
                    chunks += [(2, 3), (0, 12), (3, 3)]
            else:
                chunks = [(0, 13), (0, 14), (1, 3), (0, 15)]
            if ch < 2:
                load_xt(ch + 2)
            if ch > 0:
                for fb in range(8):
                    pending_y.append((ch - 1, fb, 0, 512))
            pops = POPS0 if ch == 0 else POPS
            for idx, (s, it) in enumerate(chunks):
                for _ in range(pops[idx]):
                    if units:
                        emit_unit(*units.popleft())
                cur = (s, it, emit_scores(s, it))
                if prev is not None:
                    emit_pv(*prev)
                prev = cur
                ny = 0 if idx < 2 else (1 if idx < 4 else 2)
                for _ in range(ny):
                    if pending_y:
                        emit_y(*pending_y.popleft())
            xt_tiles.pop(ch, None)
        emit_pv(*prev)
        for fb in range(8):
            pending_y.append((3, fb, 0, 512))
        while pending_y:
            emit_y(*pending_y.popleft())


def make_in_maps(x, w_qkv, b_qkv, w_out, b_out):
    """Host-side sharding + constant prep. Returns (in_maps, yconst_host)."""
    x = np.asarray(x, np.float32)
    w_qkv = np.asarray(w_qkv, np.float32)
    b_qkv = np.asarray(b_qkv, np.float32)
    w_out = np.asarray(w_out, np.float32)
    b_out = np.asarray(b_out, np.float32)
    bf16 = mybir.dt.np(BF16)

    slopes = (2.0 ** (-(np.arange(1, H + 1)) * 8.0 / H)).astype(np.float64)

    p = np.arange(128)[:, None]
    f128 = np.arange(128)[None, :]
    f256 = np.arange(256)[None, :]
    maskpack = (p <= np.arange(512)[None, :]).astype(bf16)

    in_maps = []
    yconst_host = np.zeros((B, D), np.float64)
    for c in range(N_CORES):
        b, j = divmod(c, 4)
        heads = [j + 4 * s for s in range(4)]
        cols = np.concatenate([np.arange(h * HD, (h + 1) * HD) for h in heads])
        wq = w_qkv[cols, :] / 8.0                  # scale folded
        wk = w_qkv[D + cols, :]
        wv = w_qkv[2 * D + cols, :]
        bq = b_qkv[cols] / 8.0
        bk = b_qkv[D + cols]
        bv = b_qkv[2 * D + cols]
        w_out_loc = w_out[:, cols]                  # [1024, 256]
        yconst_host[b] += w_out_loc.astype(np.float64) @ bv

        bias_cols = []
        for s in range(4):
            Wl = SLOT_W[s]
            sl = slopes[heads[s]]
            for o in slot_offsets(s):
                bias_cols.append(sl * (np.arange(128) - o - Wl / 2))
        bias_all = np.stack(bias_cols, axis=1).astype(np.float32)

        in_maps.append(dict(
            xT=np.ascontiguousarray(x[b].T).astype(bf16),
            wqT=np.ascontiguousarray(wq.T).astype(bf16),
            wkT=np.ascontiguousarray(wk.T).astype(bf16),
            wvT=np.ascontiguousarray(wv.T).astype(bf16),
            bqk_p=np.ascontiguousarray(
                np.stack([bq[0:128], bq[128:256],
                          bk[0:128], bk[128:256]], axis=1)),
            maskpack=maskpack, bias_all=bias_all,
            woutT=np.ascontiguousarray(
                w_out_loc.T.reshape(2, 128, D)).astype(bf16),
        ))
    yconst_host += b_out[None, :]
    return in_maps, yconst_host


_NC_CACHE = {}


def _get_nc(repeat=1):
    if repeat not in _NC_CACHE:
        _NC_CACHE[repeat] = build_nc(repeat)
    return _NC_CACHE[repeat]


def kernel(x, w_qkv, b_qkv, w_out, b_out, block_mask=None):
    in_maps, yconst_host = make_in_maps(x, w_qkv, b_qkv, w_out, b_out)
    nc = _get_nc(1)
    res = run_bass_kernel_spmd(nc, in_maps, list(range(N_CORES)), trace=False)
    y = np.zeros((B, S, D), np.float64)
    for c in range(N_CORES):
        y[c // 4] += res.results[c]["y"].astype(np.float64).T
    y += yconst_host[:, None, :]
    return y.astype(np.float32)
